# revision 28
# baseline (speedup 1.0000x reference)
"""Trainium2 Bass kernel for nn_Net_5334349382149 (4-layer GATv2 GNN + config MLP).

Sharding: destination-node partition of edges across 8 cores (2500 nodes/core),
per-layer AllGather of source features; all small stats reductions done as tiny
AllGathers (measured ~6us) + local reduce instead of AllReduce (~40us).

v2 design (vs baseline):
  - xr edge gather eliminated: per-edge XR rows come from a one-hot-transpose
    matmul against SBUF-resident node-major xr tiles
  - self-loop edges never gathered: their XL/XR rows are local SBUF tiles
  - degree-balanced node binning on host drops TPD (edge tiles per dst tile)
  - static one-hot matrices (oh01 [e,d], ohT [d,e]) built once in fp8 via K=1
    broadcast matmuls + tensor_tensor is_equal, reused by all 4 layers
  - scores via the identity  e = sum_A lrelu(h') - sum_B lrelu(-h')  computed
    by ACT-engine Lrelu(alpha=0.2) directly on PSUM ht (= ohT@xr + I@XL), with
    group-batched DVE tensor_reduce for the per-edge sums
  - no per-partition-pointer scalars on DVE (pathologically slow): biases and
    per-node scales ride the ACT engine bias/scale ports or K=1 matmuls
  - all big matmuls in bf16; xl/xr produced node-major directly (no transposes)
  - embedding lookup via one-hot matmul (no dma_gather for opcodes)
"""
import os
import sys
import numpy as np

for p in ("/opt/trn_rl_repo", "/opt/pypackages"):
    if p not in sys.path and os.path.isdir(p):
        sys.path.append(p)

import concourse.bass as bass
import concourse.tile as tile
from concourse import bacc, mybir
from concourse.masks import make_identity
from concourse.bass_utils import run_bass_kernel_spmd

F32 = mybir.dt.float32
GDT = mybir.dt.bfloat16
F8 = mybir.dt.float8e4
I16 = mybir.dt.int16
I32 = mybir.dt.int32
AF = mybir.ActivationFunctionType
ALU = mybir.AluOpType
AX = mybir.AxisListType

NCORES = 8
HID = 256
EMB = 128
OPS = 120
OPF = 140
CF = 24

CHUNK = 1024    # dma_gather rows per instruction
SGN = 4         # dst-tile groups per gather supergroup
HB = 3          # edge tiles per lrelu/reduce batch


class Cfg:
    def __init__(self, NS, NP, TPD, C, CP, nA):
        self.NS = NS            # real nodes per core
        self.NP = NP            # padded nodes per core (mult of 128)
        self.TPD = TPD          # gathered edge tiles per dst tile (self excl.)
        self.C = C              # real configs
        self.CP = CP            # padded configs (mult of 128)
        self.nA = tuple(nA)     # per-layer count of att>=0 channels
        self.NDT = NP // 128    # dst tiles per core
        self.TALL = self.NDT * TPD          # gathered edge tiles per core
        self.L = self.TALL * 128            # gather slots per core
        self.N = NS * NCORES    # total real nodes
        self.NPG = NP * NCORES  # padded global rows

    def key(self):
        return (self.NS, self.NP, self.TPD, self.C, self.CP, self.nA)


# ---------------------------------------------------------------------------
# host preprocessing
# ---------------------------------------------------------------------------

def _wrap_idx(idx):
    """int array -> int16 dma_gather layout [128, n/16] (16-wrapped, 8x repl)."""
    n = len(idx)
    assert n % 16 == 0
    w = idx.astype(np.int16).reshape(n // 16, 16).T          # [16, n/16]
    return np.tile(w, (8, 1))                                # [128, n/16]


def _balance(deg, NS, NDT):
    """Greedy LPT bin packing of nodes into NDT bins (cap 128, last cap NS-...).

    Returns pos[n] = new position of old local node n, and max bin load."""
    caps = np.full(NDT, 128, np.int64)
    caps[NDT - 1] = NS - 128 * (NDT - 1)
    order = np.argsort(-deg, kind='stable')
    loads = np.zeros(NDT, np.int64)
    counts = np.zeros(NDT, np.int64)
    pos = np.zeros(NS, np.int64)
    INF = 1 << 40
    for n in order:
        masked = np.where(counts < caps, loads, INF)
        b = int(np.argmin(masked))
        pos[n] = b * 128 + counts[b]
        counts[b] += 1
        loads[b] += deg[n]
    return pos, int(loads.max())


def host_prep(d, cfg=None):
    f32 = np.float32
    N_IN = int(np.asarray(d['node_feat']).shape[0])
    C_IN = int(np.asarray(d['config_feat']).shape[0])

    # ---- parameter folding ----
    tbl = np.asarray(d['embed_table'], f32)
    nrm = np.sqrt((tbl * tbl).sum(-1, keepdims=True))
    tbl = tbl * np.minimum(1.0, 1.0 / (nrm + 1e-7))
    W1 = np.asarray(d['early_W1'], f32)
    T1 = (tbl @ W1[:EMB]).astype(f32)                        # [OPS, 256]
    inv_std = (1.0 / (np.asarray(d['node_feat_std'], f32) + 1e-4)).astype(f32)
    w1b = (W1[EMB:] * inv_std[:, None]).astype(f32)          # [OPF, 256]
    b0 = (-(np.asarray(d['node_feat_mean'], f32) * inv_std) @ W1[EMB:]).astype(f32)

    perms, nAs = [], []
    rho = np.arange(HID)
    wl_l, wr_l, bl_l, br_l, gb_l, ratt_l = [], [], [], [], [], []
    for i in range(4):
        att = np.asarray(d['gat_att'][i], f32)
        pos_c = np.where(att >= 0)[0]
        neg_c = np.where(att < 0)[0]
        perm = np.concatenate([pos_c, neg_c])
        nAs.append(len(pos_c))
        assert np.abs(att[perm]).min() > 1e-12
        wl_l.append((np.asarray(d['gat_Wl'][i], f32)[rho][:, perm]
                     * att[perm][None, :]).astype(f32))
        wr_l.append((np.asarray(d['gat_Wr'][i], f32)[rho][:, perm]
                     * att[perm][None, :]).astype(f32))
        bl_l.append(((np.asarray(d['gat_bl'][i], f32) * att)[perm]).astype(f32))
        br_l.append(((np.asarray(d['gat_br'][i], f32) * att)[perm]).astype(f32))
        gb_l.append(np.asarray(d['gat_bias'][i], f32)[perm].astype(f32))
        ratt_l.append((1.0 / att[perm]).astype(f32))
        perms.append(perm)
        rho = perm

    cf_inv = (1.0 / (np.asarray(d['config_feat_std'], f32) + 1e-4)).astype(f32)
    LW1 = np.asarray(d['late_W1'], f32)
    w1c = (LW1[:CF] * cf_inv[:, None]).astype(f32)
    bc0 = (-(np.asarray(d['config_feat_mean'], f32) * cf_inv) @ LW1[:CF]).astype(f32)
    w1p = LW1[CF:][perms[3]].astype(f32)

    # ---- node balancing + edge sharding (self loops handled separately) ----
    NS = N_IN // NCORES
    NP_ = ((NS + 127) // 128) * 128
    NDT = NP_ // 128
    ei = np.asarray(d['edge_index']).astype(np.int64)
    src, dst = ei[0], ei[1]
    owner = dst // NS
    loc = dst % NS

    pos_all = []
    max_load = 0
    for k in range(NCORES):
        deg = np.bincount(loc[owner == k], minlength=NS)
        pos_k, ml = _balance(deg, NS, NDT)
        pos_all.append(pos_k)
        max_load = max(max_load, ml)
    tpd = max(1, (max_load + 127) // 128)
    cfg = Cfg(NS, NP_, tpd, C_IN, ((C_IN + 127) // 128) * 128, nAs)
    L = cfg.L

    src_owner = src // NS
    src_pad = np.empty_like(src)
    for k in range(NCORES):
        m = src_owner == k
        src_pad[m] = k * NP_ + pos_all[k][src[m] % NS]

    per_core = []
    for k in range(NCORES):
        m = owner == k
        sk = src_pad[m]
        newloc = pos_all[k][loc[m]]
        grp = newloc // 128
        rel = newloc % 128
        src_g = np.zeros(L, np.int64)
        dst_rel = np.full(L, -1000.0, f32)
        for g in range(NDT):
            gm = grp == g
            c = int(gm.sum())
            assert c <= tpd * 128
            base = g * tpd * 128
            src_g[base:base + c] = sk[gm]
            dst_rel[base:base + c] = rel[gm].astype(f32)
        per_core.append((src_g, dst_rel))

    # ---- per-core input maps ----
    bf16 = np.dtype('bfloat16') if hasattr(np, 'bfloat16') else None
    import ml_dtypes
    bf16 = ml_dtypes.bfloat16

    nf = np.asarray(d['node_feat'], f32)
    opc = np.asarray(d['node_opcode']).astype(np.int64)
    cfp = np.zeros((cfg.CP, CF), f32)
    cfp[:C_IN] = np.asarray(d['config_feat'], f32)

    NSV = 18
    sv = np.zeros((HID, NSV), f32)
    sv[:, 0] = b0
    for i in range(4):
        sv[:, 3 + 4 * i] = gb_l[i]
        sv[:, 4 + 4 * i] = ratt_l[i]
    sv[:, 17] = bc0

    w2 = np.asarray(d['early_W2'], f32)
    w2l = np.asarray(d['late_W2'], f32)

    def bf(x):
        return np.asarray(x, f32).astype(bf16)

    shared = {
        't1': bf(T1),
        'w1ba': bf(w1b[:128]), 'w1bb': bf(w1b[128:]),
        'w2a': bf(w2[:128]), 'w2b': bf(w2[128:]),
        'w1c': bf(w1c),
        'w1pa': bf(w1p[:128]), 'w1pb': bf(w1p[128:]),
        'w2la': bf(w2l[:128]), 'w2lb': bf(w2l[128:]),
        'predw': bf(np.asarray(d['pred_W'], f32)),
        'predb': np.asarray(d['pred_b'], f32).reshape(1, 1),
        'sv_lo': sv[:128].copy(), 'sv_hi': sv[128:].copy(),
        'cf': cfp,
    }
    for i in range(4):
        # fused [Wl | Wr] so xl and xr come from one N=512 matmul set
        shared[f'wlr{i}a'] = bf(np.concatenate([wl_l[i][:128], wr_l[i][:128]], 1))
        shared[f'wlr{i}b'] = bf(np.concatenate([wl_l[i][128:], wr_l[i][128:]], 1))
        shared[f'blrrow{i}'] = bf(np.concatenate([bl_l[i], br_l[i]]).reshape(1, 2 * HID))

    in_maps = []
    for k in range(NCORES):
        src_g, dst_rel = per_core[k]
        nfk = np.zeros((cfg.NP, OPF), f32)
        nfk[pos_all[k]] = nf[k * NS:(k + 1) * NS]
        ok = np.zeros(cfg.NP, np.int64)
        ok[pos_all[k]] = opc[k * NS:(k + 1) * NS]
        m = dict(shared)
        m['nf'] = nfk
        m['opcol'] = ok.reshape(NDT, 128).astype(f32).T.copy()
        m['drelcols'] = dst_rel.reshape(cfg.TALL, 128).T.copy()
        m['srcidx'] = _wrap_idx(src_g)
        in_maps.append(m)
    return cfg, in_maps


# ---------------------------------------------------------------------------
# program builder
# ---------------------------------------------------------------------------

def build_program(cfg: Cfg):
    nc = bacc.Bacc("TRN2", target_bir_lowering=False, debug=False,
                   num_devices=NCORES, num_swdge_queues=2)
    NP_, NS, TPD, NDT, L = cfg.NP, cfg.NS, cfg.TPD, cfg.NDT, cfg.L
    NT = NDT
    TALL = cfg.TALL
    REPL = [list(range(NCORES))]

    def din(name, shape, dt=F32):
        return nc.dram_tensor(name, list(shape), dt, kind="ExternalInput")

    # ---- external inputs ----
    nf_d = din('nf', (NP_, OPF))
    t1_d = din('t1', (OPS, HID), GDT)
    opcol_d = din('opcol', (128, NDT))
    drelcols_d = din('drelcols', (128, TALL))
    srcidx_d = din('srcidx', (128, L // 16), I16)
    w1ba_d = din('w1ba', (128, HID), GDT)
    w1bb_d = din('w1bb', (OPF - 128, HID), GDT)
    w2_d = [din('w2a', (128, HID), GDT), din('w2b', (128, HID), GDT)]
    wlr_d = [[din(f'wlr{i}a', (128, 2 * HID), GDT),
              din(f'wlr{i}b', (128, 2 * HID), GDT)] for i in range(4)]
    blr_d = [din(f'blrrow{i}', (1, 2 * HID), GDT) for i in range(4)]
    w1c_d = din('w1c', (CF, HID), GDT)
    w1p_d = [din('w1pa', (128, HID), GDT), din('w1pb', (128, HID), GDT)]
    w2l_d = [din('w2la', (128, 128), GDT), din('w2lb', (128, 128), GDT)]
    predw_d = din('predw', (128, 1), GDT)
    predb_d = din('predb', (1, 1))
    sv_d = [din('sv_lo', (128, 18)), din('sv_hi', (128, 18))]
    cf_d = din('cf', (cfg.CP, CF))
    out_d = nc.dram_tensor('out', [1, cfg.CP], F32, kind="ExternalOutput")

    # ---- internal DRAM ----
    xl_own = [nc.dram_tensor(f'xl_own{i}', [NP_, HID], GDT) for i in range(4)]
    xl_full = [nc.dram_tensor(f'xl_full{i}', [cfg.NPG, HID], GDT,
                              addr_space="Shared") for i in range(4)]
    ar_in = [nc.dram_tensor(f'ar_in{i}', [128, 4], F32) for i in range(6)]
    ar_out = [nc.dram_tensor(f'ar_out{i}', [128 * NCORES, 4], F32,
                             addr_space="Shared") for i in range(6)]
    pool_in = nc.dram_tensor('pool_in', [128, 4], F32)
    pool_out = nc.dram_tensor('pool_out', [128 * NCORES, 4], F32,
                              addr_space="Shared")

    with tile.TileContext(nc) as tc, __import__('contextlib').ExitStack() as ctx:
        const = ctx.enter_context(tc.tile_pool(name="const", bufs=1))
        big = ctx.enter_context(tc.tile_pool(name="big", bufs=1))
        work = ctx.enter_context(tc.tile_pool(name="work", bufs=3))
        col = ctx.enter_context(tc.tile_pool(name="col", bufs=6))

        # ------ constants ------
        ident = const.tile([128, 128], F32, tag="ident", name="ident")
        make_identity(nc, ident[:])
        ident_bf = const.tile([128, 128], GDT, tag="identbf", name="identbf")
        nc.vector.tensor_copy(ident_bf[:], ident[:])
        iota_i = const.tile([128, 128], I32, tag="iota_i", name="iota_i")
        nc.gpsimd.iota(iota_i[:], pattern=[[1, 128]], base=0, channel_multiplier=0)
        iota_f = const.tile([128, 128], F32, tag="iota_f", name="iota_f")
        nc.vector.tensor_copy(iota_f[:], iota_i[:])
        ones_row = const.tile([1, 512], GDT, tag="onesrow", name="onesrow")
        nc.gpsimd.memset(ones_row[:], 1.0)
        ones_bf = const.tile([128, 1], GDT, tag="onesbf", name="onesbf")
        nc.gpsimd.memset(ones_bf[:], 1.0)
        zero_col = const.tile([128, 1], F32, tag="zeroc", name="zeroc")
        nc.gpsimd.memset(zero_col[:], 0.0)
        nc.const_aps.aps[(F32, 0.0)] = zero_col[:]
        eps_col = const.tile([128, 1], F32, tag="epsc", name="epsc")
        nc.gpsimd.memset(eps_col[:], 1e-5)

        def load_const(dram, tag):
            t = const.tile(list(dram.shape), dram.dtype, tag=tag)
            nc.sync.dma_start(out=t[:], in_=dram[:])
            return t

        srcidx = load_const(srcidx_d, 'srcidx')
        opcol = load_const(opcol_d, 'opcol')
        drelcols = load_const(drelcols_d, 'drelcols')
        t1 = load_const(t1_d, 't1')
        w1ba = load_const(w1ba_d, 'w1ba')
        w1bb = load_const(w1bb_d, 'w1bb')
        w2 = [load_const(w2_d[j], f'w2{j}') for j in range(2)]
        wlr = [[load_const(wlr_d[i][j], f'wlr{i}{j}') for j in range(2)]
               for i in range(4)]
        blr = [load_const(blr_d[i], f'blr{i}') for i in range(4)]
        w1c = load_const(w1c_d, 'w1c')
        w1p = [load_const(w1p_d[j], f'w1p{j}') for j in range(2)]
        w2l = [load_const(w2l_d[j], f'w2l{j}') for j in range(2)]
        predw = load_const(predw_d, 'predw')
        predb = load_const(predb_d, 'predb')
        sv = [load_const(sv_d[j], f'sv{j}') for j in range(2)]

        # ------ persistent big tiles ------
        raw = [big.tile([128, NP_], F32, tag=f"raw{m}", name=f"raw{m}") for m in range(2)]
        xt = [big.tile([128, NP_], GDT, tag=f"x{m}", name=f"x{m}") for m in range(2)]
        xl_sb = big.tile([128, NDT * HID], GDT, tag="xlsb", name="xlsb")
        xr_sb = big.tile([128, NDT * HID], GDT, tag="xrsb", name="xrsb")
        oh01 = big.tile([128, TALL * 128], F8, tag="oh01", name="oh01")
        ohT = big.tile([128, TALL * 128], F8, tag="ohT", name="ohT")

        blocks = [(s, min(s + 512, NP_)) for s in range(0, NP_, 512)]

        def stats_tiles(tagp):
            return [work.tile([128, max(len(blocks), NDT)], F32,
                              tag=f"st{m}", name=f"{tagp}{m}") for m in range(2)]

        sqscr = big.tile([128, 512], F32, tag="sqscr", name="sqscr")

        # --- evac: dst[:, c0:c1] = src*scale + bias, stats over cols < NS ---
        def evac_cm(dst, src_ap, c0, c1, mc, st1, st2, bi, bias_col, scale_col,
                    nreal=NS):
            def one(a, b, accum):
                kw = {'accum_out': st1[mc][:, bi:bi + 1]} if accum else {}
                nc.scalar.activation(dst[:, a:b], src_ap[:, a - c0:b - c0],
                                     AF.Identity, bias=bias_col, scale=scale_col,
                                     **kw)
                if accum:
                    nc.vector.scalar_tensor_tensor(
                        sqscr[:, :b - a], dst[:, a:b], 1.0, dst[:, a:b],
                        ALU.mult, ALU.mult,
                        accum_out=st2[mc][:, bi:bi + 1])
            if c0 >= nreal:
                one(c0, c1, False)
            elif c1 <= nreal:
                one(c0, c1, True)
            else:
                one(c0, nreal, True)
                one(nreal, c1, False)

        # --- stats exchange via tiny AllGather + local reduce ---
        def stats_allgather(st1, st2, ar_i, ar_o, ntotal, nblk):
            art = work.tile([128, 4], F32, tag="art", name="art")
            for m in range(2):
                nc.vector.tensor_reduce(art[:, 2 * m:2 * m + 1], st1[m][:, :nblk],
                                        AX.X, ALU.add)
                nc.vector.tensor_reduce(art[:, 2 * m + 1:2 * m + 2], st2[m][:, :nblk],
                                        AX.X, ALU.add)
            nc.sync.dma_start(out=ar_i[:], in_=art[:])
            nc.gpsimd.collective_compute(
                "AllGather", ALU.bypass, replica_groups=REPL,
                ins=[ar_i[:]], outs=[ar_o[:]])
            arr = work.tile([128, NCORES * 4], F32, tag="arrg", name="arrg")
            nc.sync.dma_start(out=arr[:].rearrange("p (k v) -> p k v", v=4),
                              in_=ar_o[:].rearrange("(k p) v -> p k v", p=128))
            arr3 = arr[:].rearrange("p (k v) -> p k v", v=4)
            rs_l, nmr_l = [], []
            for m in range(2):
                s1 = col.tile([128, 1], F32, tag="cs1", name="cs1")
                s2 = col.tile([128, 1], F32, tag="cs2", name="cs2")
                nc.vector.tensor_reduce(s1[:], arr3[:, :, 2 * m], AX.X, ALU.add)
                nc.vector.tensor_reduce(s2[:], arr3[:, :, 2 * m + 1], AX.X, ALU.add)
                rs, nmr = _finish_stats(s1, s2, ntotal)
                rs_l.append(rs)
                nmr_l.append(nmr)
            return rs_l, nmr_l

        def _finish_stats(s1, s2, ntotal):
            mu = col.tile([128, 1], F32, tag="mu", name="mu")
            nc.vector.tensor_scalar(mu[:], s1[:], 1.0 / ntotal, None, ALU.mult)
            mu2 = col.tile([128, 1], F32, tag="mu2", name="mu2")
            nc.scalar.activation(mu2[:], mu[:], AF.Square)
            var = col.tile([128, 1], F32, tag="var", name="var")
            nc.vector.scalar_tensor_tensor(var[:], s2[:], 1.0 / ntotal, mu2[:],
                                           ALU.mult, ALU.subtract)
            sd = col.tile([128, 1], F32, tag="sd", name="sd")
            nc.scalar.activation(sd[:], var[:], AF.Sqrt, bias=eps_col[:])
            rs = col.tile([128, 1], F32, tag="rs", name="rs")
            nc.vector.reciprocal(rs[:], sd[:])
            t = col.tile([128, 1], F32, tag="nmrt", name="nmrt")
            nc.vector.tensor_tensor(t[:], mu[:], rs[:], ALU.mult)
            nmr = col.tile([128, 1], F32, tag="nmr", name="nmr")
            nc.vector.tensor_scalar(nmr[:], t[:], -1.0, None, ALU.mult)
            return rs, nmr

        def norm_gelu(src_tiles, dst_tiles, rs_l, nmr_l):
            for m in range(2):
                nc.scalar.activation(dst_tiles[m][:], src_tiles[m][:], AF.Gelu,
                                     bias=nmr_l[m][:], scale=rs_l[m][:])

        # =================== early stage ===================
        early = tc.alloc_tile_pool(name="early", bufs=1)
        ps_early = tc.alloc_tile_pool(name="pse", bufs=2, space="PSUM")
        nfTa = early.tile([128, NP_], GDT, tag="nfTa", name="nfTa")
        nfTb = early.tile([OPF - 128, NP_], GDT, tag="nfTb", name="nfTb")
        ohop = early.tile([OPS, NP_], GDT, tag="ohop", name="ohop")
        nf_nm = early.tile([128, NT * OPF], F32, tag="nf_nm", name="nf_nm")
        nc.sync.dma_start(
            out=nf_nm[:].rearrange("p (t c) -> p t c", c=OPF),
            in_=nf_d[:].rearrange("(t p) c -> p t c", p=128))

        for t in range(NT):
            ps = ps_early.tile([128, 512], F32, tag="mm", name="etr")
            nc.tensor.matmul(ps[:, :128], lhsT=nf_nm[:, t * OPF:t * OPF + 128],
                             rhs=ident[:], is_transpose=True, start=True, stop=True)
            nc.vector.tensor_copy(nfTa[:, t * 128:(t + 1) * 128], ps[:, :128])
            ps2 = ps_early.tile([128, 512], F32, tag="mm", name="etr2")
            nc.tensor.matmul(ps2[:OPF - 128, :128],
                             lhsT=nf_nm[:, t * OPF + 128:(t + 1) * OPF],
                             rhs=ident[:], is_transpose=True, start=True, stop=True)
            nc.vector.tensor_copy(nfTb[:, t * 128:(t + 1) * 128],
                                  ps2[:OPF - 128, :128])
            # opcode one-hot (op on partitions): ohop[op, n] = (opc[n] == op)
            opsb = work.tile([128, OPS], F32, tag="opsb", name="opsb")
            nc.scalar.activation(opsb[:], iota_f[:, :OPS], AF.Identity,
                                 bias=opcol[:, t:t + 1], scale=0.0)
            ohnf = work.tile([128, OPS], F32, tag="ohnf", name="ohnf")
            nc.vector.tensor_tensor(ohnf[:], opsb[:], iota_f[:, :OPS],
                                    ALU.is_equal)
            ps3 = ps_early.tile([128, 512], F32, tag="mm", name="eoh")
            nc.tensor.matmul(ps3[:OPS, :128], lhsT=ohnf[:], rhs=ident[:],
                             is_transpose=True, start=True, stop=True)
            nc.vector.tensor_copy(ohop[:, t * 128:(t + 1) * 128], ps3[:OPS, :128])

        # early layer 1: raw = nf @ w1b + T1[op] + b0   (channel-major)
        st1 = stats_tiles("e1s1")
        st2 = stats_tiles("e1s2")
        for mc in range(2):
            for bi, (s, e) in enumerate(blocks):
                w = e - s
                ps = ps_early.tile([128, 512], F32, tag="mm", name="mm")
                nc.tensor.matmul(ps[:, :w], lhsT=w1ba[:, mc * 128:(mc + 1) * 128],
                                 rhs=nfTa[:, s:e], start=True, stop=False)
                nc.tensor.matmul(ps[:, :w], lhsT=w1bb[:, mc * 128:(mc + 1) * 128],
                                 rhs=nfTb[:, s:e], start=False, stop=False)
                nc.tensor.matmul(ps[:, :w], lhsT=t1[:, mc * 128:(mc + 1) * 128],
                                 rhs=ohop[:, s:e], start=False, stop=True)
                evac_cm(raw[mc], ps[:, :w], s, e, mc, st1, st2, bi,
                        sv[mc][:, 0:1], 1.0)
        rs_l, nmr_l = stats_allgather(st1, st2, ar_in[0], ar_out[0], cfg.N,
                                      len(blocks))
        norm_gelu(raw, xt, rs_l, nmr_l)

        # early layer 2: raw = x @ w2
        st1 = stats_tiles("e2s1")
        st2 = stats_tiles("e2s2")
        for mc in range(2):
            for bi, (s, e) in enumerate(blocks):
                w = e - s
                ps = ps_early.tile([128, 512], F32, tag="mm", name="mm")
                for kc in range(2):
                    nc.tensor.matmul(ps[:, :w], lhsT=w2[kc][:, mc * 128:(mc + 1) * 128],
                                     rhs=xt[kc][:, s:e], start=(kc == 0),
                                     stop=(kc == 1))
                evac_cm(raw[mc], ps[:, :w], s, e, mc, st1, st2, bi,
                        zero_col[:], 1.0)
        rs_l, nmr_l = stats_allgather(st1, st2, ar_in[1], ar_out[1], cfg.N,
                                      len(blocks))
        norm_gelu(raw, xt, rs_l, nmr_l)
        early.release()
        ps_early.release()

        # =================== static one-hots (all layers) ===================
        ps_oh = tc.alloc_tile_pool(name="psoh", bufs=2, space="PSUM")
        for t in range(TALL):
            dsb = work.tile([128, 128], F32, tag="dsb", name="dsb")
            nc.scalar.activation(dsb[:], iota_f[:], AF.Identity,
                                 bias=drelcols[:, t:t + 1], scale=0.0)
            ohf = work.tile([128, 128], F32, tag="ohf", name="ohf")
            nc.vector.tensor_tensor(ohf[:], dsb[:], iota_f[:], ALU.is_equal)
            nc.vector.tensor_copy(oh01[:, t * 128:(t + 1) * 128], ohf[:])
            psr = ps_oh.tile([128, 128], F32, tag="dc", name="dr")
            nc.tensor.matmul(psr[:], lhsT=ohf[:], rhs=ident[:],
                             is_transpose=True, start=True, stop=True)
            nc.vector.tensor_copy(ohT[:, t * 128:(t + 1) * 128], psr[:])
        ps_oh.release()

        # =================== GAT layers ===================
        gat = tc.alloc_tile_pool(name="gath", bufs=2)
        ps_ht = tc.alloc_tile_pool(name="psht", bufs=2, space="PSUM")
        ps_agg = tc.alloc_tile_pool(name="psagg", bufs=2, space="PSUM")
        ps_sm = tc.alloc_tile_pool(name="pssm", bufs=2, space="PSUM")
        SGT = SGN * TPD                      # gathered tiles per supergroup
        NSG = (NDT + SGN - 1) // SGN

        for li in range(4):
            nA = cfg.nA[li]
            # ---- fused xl|xr node-major (bias folded in as K=1 matmul) ----
            for t in range(NT):
                ps = ps_sm.tile([128, 512], F32, tag="nm", name="nm")
                for kc in range(2):
                    nc.tensor.matmul(
                        ps[:], lhsT=xt[kc][:, t * 128:(t + 1) * 128],
                        rhs=wlr[li][kc][:], start=(kc == 0), stop=False)
                nc.tensor.matmul(ps[:], lhsT=ones_row[:1, :128],
                                 rhs=blr[li][:1, :], start=False, stop=True)
                nc.vector.tensor_copy(xl_sb[:, t * HID:(t + 1) * HID], ps[:, :HID])
                nc.vector.tensor_copy(xr_sb[:, t * HID:(t + 1) * HID], ps[:, HID:])
            nc.sync.dma_start(
                out=xl_own[li][:].rearrange("(t p) c -> p t c", p=128),
                in_=xl_sb[:].rearrange("p (t c) -> p t c", c=HID))
            nc.gpsimd.collective_compute(
                "AllGather", ALU.bypass, replica_groups=REPL,
                ins=[xl_own[li][:]], outs=[xl_full[li][:]])

            # ---- edge processing (agg phase pipelined one group behind) ----
            st1 = stats_tiles("gs1")
            st2 = stats_tiles("gs2")
            TT = TPD + 1                      # gathered + self

            def score_phase(g, xlg, base_t):
                xl_g = xl_sb[:, g * HID:(g + 1) * HID]
                xr_g = xr_sb[:, g * HID:(g + 1) * HID]
                ecols = col.tile([128, 16], F32, tag="ecols", name="ecols")
                for h0 in range(0, TT, HB):
                    hbn = min(HB, TT - h0)
                    htp = ps_ht.tile([128, HB * HID], F32, tag="ht", name="ht")
                    for jj in range(hbn):
                        j = h0 + jj
                        sl = htp[:, jj * HID:(jj + 1) * HID]
                        if j < TPD:
                            tix = g * TPD + j
                            nc.tensor.matmul(
                                sl, lhsT=ohT[:, tix * 128:(tix + 1) * 128],
                                rhs=xr_g, start=True, stop=False)
                            nc.tensor.matmul(
                                sl, lhsT=ident_bf[:],
                                rhs=xlg[:, (base_t + j) * HID:(base_t + j + 1) * HID],
                                start=False, stop=True)
                        else:
                            nc.tensor.matmul(sl, lhsT=ident_bf[:], rhs=xr_g,
                                             start=True, stop=False)
                            nc.tensor.matmul(sl, lhsT=ident_bf[:], rhs=xl_g,
                                             start=False, stop=True)
                    scr = work.tile([128, HB * HID], GDT, tag="scr", name="scr")
                    h3 = htp[:, :hbn * HID].rearrange("p (t c) -> p t c", c=HID)
                    s3 = scr[:, :hbn * HID].rearrange("p (t c) -> p t c", c=HID)
                    nc.scalar.activation(s3[:, :, :nA], h3[:, :, :nA], AF.Lrelu,
                                         alpha=0.2)
                    nc.scalar.activation(s3[:, :, nA:], h3[:, :, nA:], AF.Lrelu,
                                         scale=-1.0, alpha=0.2)
                    ra = col.tile([128, HB], F32, tag="ra", name="ra")
                    rb = col.tile([128, HB], F32, tag="rb", name="rb")
                    nc.vector.tensor_reduce(ra[:, :hbn], s3[:, :, :nA], AX.X,
                                            ALU.add)
                    nc.vector.tensor_reduce(rb[:, :hbn], s3[:, :, nA:], AX.X,
                                            ALU.add)
                    nc.vector.tensor_tensor(ecols[:, h0:h0 + hbn], ra[:, :hbn],
                                            rb[:, :hbn], ALU.subtract)
                wcols = col.tile([128, 16], F32, tag="wcols", name="wcols")
                nc.scalar.activation(wcols[:, :TT], ecols[:, :TT], AF.Exp)
                return wcols

            def agg_phase(g, xlg, base_t, wcols):
                xl_g = xl_sb[:, g * HID:(g + 1) * HID]
                agg = ps_agg.tile([128, 512], F32, tag="agg", name="agg")
                ps256 = agg[:, :HID]
                ps1 = agg[:, HID:HID + 1]
                for j in range(TPD):
                    tix = g * TPD + j
                    ohs = work.tile([128, 128], GDT, tag="ohs", name="ohs")
                    nc.scalar.activation(ohs[:], oh01[:, tix * 128:(tix + 1) * 128],
                                         AF.Copy, scale=wcols[:, j:j + 1])
                    nc.tensor.matmul(ps256, lhsT=ohs[:],
                                     rhs=xlg[:, (base_t + j) * HID:(base_t + j + 1) * HID],
                                     start=(j == 0), stop=(j == TPD - 1))
                    nc.tensor.matmul(ps1, lhsT=ohs[:], rhs=ones_bf[:],
                                     start=(j == 0), stop=(j == TPD - 1))
                # ---- evacuation ----
                wself = wcols[:, TPD:TPD + 1]
                dcol = col.tile([128, 1], F32, tag="dcol", name="dcol")
                nc.vector.scalar_tensor_tensor(dcol[:], ps1, 1e-16, wself,
                                               ALU.add, ALU.add)
                rcol = col.tile([128, 1], F32, tag="rcol", name="rcol")
                nc.vector.reciprocal(rcol[:], dcol[:])
                smsg = work.tile([128, HID], F32, tag="smsg", name="smsg")
                nc.scalar.activation(smsg[:], xl_g, AF.Copy, scale=wself)
                msum = work.tile([128, HID], F32, tag="msum", name="msum")
                nc.vector.scalar_tensor_tensor(msum[:], ps256, 1.0, smsg[:],
                                               ALU.mult, ALU.add)
                msg = work.tile([128, HID], F32, tag="msg", name="msg")
                nc.scalar.activation(msg[:], msum[:], AF.Copy, scale=rcol[:])
                for mc in range(2):
                    pstr = ps_sm.tile([128, 512], F32, tag="nm", name="tr")
                    nc.tensor.matmul(pstr[:, :128],
                                     lhsT=msg[:, mc * 128:(mc + 1) * 128],
                                     rhs=ident[:], is_transpose=True,
                                     start=True, stop=True)
                    evac_cm(raw[mc], pstr[:, :128], g * 128, (g + 1) * 128, mc,
                            st1, st2, g, sv[mc][:, 3 + 4 * li:4 + 4 * li],
                            sv[mc][:, 4 + 4 * li:5 + 4 * li])

            xlg = None
            qn = [0]
            prev = None
            for g in range(NDT):
                if g % SGN == 0:
                    sg = g // SGN
                    ntiles = min(SGT, TALL - sg * SGT)
                    xlg = gat.tile([128, SGT * HID], GDT, tag="xlg", name="xlg")
                    total = ntiles * 128
                    i0 = sg * SGT * 128
                    done = 0
                    x3 = xlg[:].rearrange("p (t c) -> p t c", c=HID)
                    while done < total:
                        n = min(CHUNK, total - done)
                        nc.gpsimd.dma_gather(
                            out_ap=x3[:, done // 128:(done + n) // 128, :],
                            in_ap=xl_full[li][:],
                            idxs_ap=srcidx[:, (i0 + done) // 16:(i0 + done + n) // 16],
                            num_idxs=n, num_idxs_reg=n, elem_size=HID,
                            queue_num=qn[0] % 2)
                        qn[0] += 1
                        done += n
                base_t = (g % SGN) * TPD
                wc = score_phase(g, xlg, base_t)
                if prev is not None:
                    agg_phase(*prev)
                prev = (g, xlg, base_t, wc)
            agg_phase(*prev)
            rs_l, nmr_l = stats_allgather(st1, st2, ar_in[2 + li], ar_out[2 + li],
                                          cfg.N, NDT)
            norm_gelu(raw, xt, rs_l, nmr_l)

        ps_sm.release()
        ps_agg.release()
        ps_ht.release()
        gat.release()

        # =================== pooling ===================
        pt = work.tile([128, 4], F32, tag="pt", name="pt")
        for m in range(2):
            nc.vector.tensor_reduce(pt[:, m:m + 1], xt[m][:, :NS], AX.X, ALU.add)
            nc.vector.tensor_reduce(pt[:, 2 + m:3 + m], xt[m][:, :NS], AX.X, ALU.max)
        nc.sync.dma_start(out=pool_in[:], in_=pt[:])
        nc.gpsimd.collective_compute(
            "AllGather", ALU.bypass, replica_groups=REPL,
            ins=[pool_in[:]], outs=[pool_out[:]])
        pg = work.tile([128, NCORES * 4], F32, tag="pg", name="pg")
        nc.sync.dma_start(out=pg[:].rearrange("p (k v) -> p k v", v=4),
                          in_=pool_out[:].rearrange("(k p) v -> p k v", p=128))
        pg3 = pg[:].rearrange("p (k v) -> p k v", v=4)
        pool_c = []
        for m in range(2):
            s_ = col.tile([128, 1], F32, tag="psum_c", name="psum_c")
            nc.vector.tensor_reduce(s_[:], pg3[:, :, m], AX.X, ALU.add)
            mx = col.tile([128, 1], F32, tag="pmax_c", name="pmax_c")
            nc.vector.tensor_reduce(mx[:], pg3[:, :, 2 + m], AX.X, ALU.max)
            pc = col.tile([128, 1], GDT, tag="pool_c", name="pool_c")
            pcf = col.tile([128, 1], F32, tag="pool_cf", name="pool_cf")
            nc.vector.scalar_tensor_tensor(pcf[:], s_[:], 1.0 / cfg.N, mx[:],
                                           ALU.mult, ALU.add)
            nc.vector.tensor_copy(pc[:], pcf[:])
            pool_c.append(pc)

        # =================== late MLP (replicated) ===================
        cblocks = [(s, min(s + 512, cfg.CP)) for s in range(0, cfg.CP, 512)]
        NCT = cfg.CP // 128
        late = tc.alloc_tile_pool(name="late", bufs=1)
        ps_late = tc.alloc_tile_pool(name="psl", bufs=2, space="PSUM")
        cf_nm = work.tile([128, NCT * CF], F32, tag="cf_nm", name="cf_nm")
        nc.sync.dma_start(out=cf_nm[:].rearrange("p (t c) -> p t c", c=CF),
                          in_=cf_d[:].rearrange("(t p) c -> p t c", p=128))
        cfT = late.tile([CF, cfg.CP], GDT, tag="cfT", name="cfT")
        for t in range(NCT):
            ps = ps_late.tile([128, 512], F32, tag="mm", name="ctr")
            nc.tensor.matmul(ps[:CF, :128], lhsT=cf_nm[:, t * CF:(t + 1) * CF],
                             rhs=ident[:], is_transpose=True, start=True, stop=True)
            nc.vector.tensor_copy(cfT[:, t * 128:(t + 1) * 128], ps[:CF, :128])

        # vec1 = w1p^T @ pool  (+ bc0)
        vcol = []
        for mc in range(2):
            ps = ps_late.tile([128, 512], F32, tag="v1", name="v1")
            for kc in range(2):
                nc.tensor.matmul(ps[:, :1], lhsT=w1p[kc][:, mc * 128:(mc + 1) * 128],
                                 rhs=pool_c[kc][:], start=(kc == 0), stop=(kc == 1))
            v = col.tile([128, 1], F32, tag="vcol", name="vcol")
            nc.vector.scalar_tensor_tensor(v[:], sv[mc][:, 17:18], 1.0, ps[:, :1],
                                           ALU.mult, ALU.add)
            vcol.append(v)

        h1r = [late.tile([128, cfg.CP], F32, tag=f"h1r{m}", name=f"h1r{m}")
               for m in range(2)]
        h1 = [late.tile([128, cfg.CP], GDT, tag=f"h1_{m}", name=f"h1_{m}")
              for m in range(2)]
        h2r = late.tile([128, cfg.CP], F32, tag="h2r", name="h2r")
        h2 = late.tile([128, cfg.CP], GDT, tag="h2_", name="h2_")

        def late_stats_norm(src_tiles, dst_tiles, st1, st2, nblk, nmc):
            for m in range(nmc):
                s1 = col.tile([128, 1], F32, tag="cs1", name="cs1")
                s2 = col.tile([128, 1], F32, tag="cs2", name="cs2")
                nc.vector.tensor_reduce(s1[:], st1[m][:, :nblk], AX.X, ALU.add)
                nc.vector.tensor_reduce(s2[:], st2[m][:, :nblk], AX.X, ALU.add)
                rs, nmr = _finish_stats(s1, s2, cfg.C)
                nc.scalar.activation(dst_tiles[m][:], src_tiles[m][:], AF.Gelu,
                                     bias=nmr[:], scale=rs[:])

        # h1 = gelu(cfgnorm(cf @ w1c + vec1))
        st1 = stats_tiles("l1s1")
        st2 = stats_tiles("l1s2")
        for mc in range(2):
            for bi, (s, e) in enumerate(cblocks):
                w = e - s
                ps = ps_late.tile([128, 512], F32, tag="mm", name="mm")
                nc.tensor.matmul(ps[:, :w], lhsT=w1c[:, mc * 128:(mc + 1) * 128],
                                 rhs=cfT[:, s:e], start=True, stop=True)
                evac_cm(h1r[mc], ps[:, :w], s, e, mc, st1, st2, bi,
                        vcol[mc][:], 1.0, nreal=cfg.C)
        late_stats_norm(h1r, h1, st1, st2, len(cblocks), 2)

        # h2 = gelu(cfgnorm(h1 @ w2l))   (128 out channels -> mc=0 only)
        st1 = stats_tiles("l2s1")
        st2 = stats_tiles("l2s2")
        for bi, (s, e) in enumerate(cblocks):
            w = e - s
            ps = ps_late.tile([128, 512], F32, tag="mm", name="mm")
            for kc in range(2):
                nc.tensor.matmul(ps[:, :w], lhsT=w2l[kc][:], rhs=h1[kc][:, s:e],
                                 start=(kc == 0), stop=(kc == 1))
            evac_cm(h2r, ps[:, :w], s, e, 0, st1, st2, bi, zero_col[:], 1.0,
                    nreal=cfg.C)
        late_stats_norm([h2r], [h2], st1, st2, len(cblocks), 1)

        # pred: out = h2^T @ predw + predb
        outsb = work.tile([1, cfg.CP], F32, tag="outsb", name="outsb")
        for (s, e) in cblocks:
            w = e - s
            ps = ps_late.tile([128, 512], F32, tag="mm", name="predps")
            nc.tensor.matmul(ps[:1, :w], lhsT=predw[:], rhs=h2[:, s:e],
                             start=True, stop=True)
            nc.vector.tensor_scalar(outsb[:, s:e], ps[:1, :w], predb[:],
                                    None, ALU.add)
        nc.sync.dma_start(out=out_d[:], in_=outsb[:])
        late.release()
        ps_late.release()

    nc.compile()
    return nc


# ---------------------------------------------------------------------------
# entry point
# ---------------------------------------------------------------------------

_prog_cache = {}


def kernel(**inputs) -> np.ndarray:
    cfg, in_maps = host_prep(inputs)
    key = cfg.key()
    if key not in _prog_cache:
        _prog_cache[key] = build_program(cfg)
    nc = _prog_cache[key]
    res = run_bass_kernel_spmd(nc, in_maps, list(range(NCORES)))
    out = np.asarray(res.results[0]['out']).reshape(-1)[:cfg.C]
    return out.astype(np.float32)


# revision 30
# speedup vs baseline: 1.1981x; 1.1981x over previous
"""Trainium2 Bass kernel for nn_Net_5334349382149 (4-layer GATv2 GNN + config MLP).

Sharding: destination-node partition of edges across 8 cores (2500 nodes/core),
per-layer AllGather of source features; all small stats reductions done as tiny
AllGathers (measured ~6us) + local reduce instead of AllReduce (~40us).

v2 design (vs baseline):
  - xr edge gather eliminated: per-edge XR rows come from a one-hot-transpose
    matmul against SBUF-resident node-major xr tiles
  - self-loop edges never gathered: their XL/XR rows are local SBUF tiles
  - degree-balanced node binning on host drops TPD (edge tiles per dst tile)
  - static one-hot matrices (oh01 [e,d], ohT [d,e]) built once in fp8 via K=1
    broadcast matmuls + tensor_tensor is_equal, reused by all 4 layers
  - scores via the identity  e = sum_A lrelu(h') - sum_B lrelu(-h')  computed
    by ACT-engine Lrelu(alpha=0.2) directly on PSUM ht (= ohT@xr + I@XL), with
    group-batched DVE tensor_reduce for the per-edge sums
  - no per-partition-pointer scalars on DVE (pathologically slow): biases and
    per-node scales ride the ACT engine bias/scale ports or K=1 matmuls
  - all big matmuls in bf16; xl/xr produced node-major directly (no transposes)
  - embedding lookup via one-hot matmul (no dma_gather for opcodes)
"""
import os
import sys
import numpy as np

for p in ("/opt/trn_rl_repo", "/opt/pypackages"):
    if p not in sys.path and os.path.isdir(p):
        sys.path.append(p)

import concourse.bass as bass
import concourse.tile as tile
from concourse import bacc, mybir
from concourse.masks import make_identity
from concourse.bass_utils import run_bass_kernel_spmd

F32 = mybir.dt.float32
GDT = mybir.dt.bfloat16
F8 = mybir.dt.float8e4
I16 = mybir.dt.int16
I32 = mybir.dt.int32
AF = mybir.ActivationFunctionType
ALU = mybir.AluOpType
AX = mybir.AxisListType

NCORES = 8
HID = 256
EMB = 128
OPS = 120
OPF = 140
CF = 24

CHUNK = 1024    # dma_gather rows per instruction
SGN = 4         # dst-tile groups per gather supergroup
HB = 2          # edge tiles per lrelu/reduce batch


class Cfg:
    def __init__(self, NS, NP, TPD, C, CP, nA):
        self.NS = NS            # real nodes per core
        self.NP = NP            # padded nodes per core (mult of 128)
        self.TPD = TPD          # gathered edge tiles per dst tile (self excl.)
        self.C = C              # real configs
        self.CP = CP            # padded configs (mult of 128)
        self.nA = tuple(nA)     # per-layer count of att>=0 channels
        self.NDT = NP // 128    # dst tiles per core
        self.TALL = self.NDT * TPD          # gathered edge tiles per core
        self.L = self.TALL * 128            # gather slots per core
        self.N = NS * NCORES    # total real nodes
        self.NPG = NP * NCORES  # padded global rows

    def key(self):
        return (self.NS, self.NP, self.TPD, self.C, self.CP, self.nA)


# ---------------------------------------------------------------------------
# host preprocessing
# ---------------------------------------------------------------------------

def _wrap_idx(idx):
    """int array -> int16 dma_gather layout [128, n/16] (16-wrapped, 8x repl)."""
    n = len(idx)
    assert n % 16 == 0
    w = idx.astype(np.int16).reshape(n // 16, 16).T          # [16, n/16]
    return np.tile(w, (8, 1))                                # [128, n/16]


def _balance(deg, NS, NDT):
    """Greedy LPT bin packing of nodes into NDT bins (cap 128, last cap NS-...).

    Returns pos[n] = new position of old local node n, and max bin load."""
    caps = np.full(NDT, 128, np.int64)
    caps[NDT - 1] = NS - 128 * (NDT - 1)
    order = np.argsort(-deg, kind='stable')
    loads = np.zeros(NDT, np.int64)
    counts = np.zeros(NDT, np.int64)
    pos = np.zeros(NS, np.int64)
    INF = 1 << 40
    for n in order:
        masked = np.where(counts < caps, loads, INF)
        b = int(np.argmin(masked))
        pos[n] = b * 128 + counts[b]
        counts[b] += 1
        loads[b] += deg[n]
    return pos, int(loads.max())


def host_prep(d, cfg=None):
    f32 = np.float32
    N_IN = int(np.asarray(d['node_feat']).shape[0])
    C_IN = int(np.asarray(d['config_feat']).shape[0])

    # ---- parameter folding ----
    tbl = np.asarray(d['embed_table'], f32)
    nrm = np.sqrt((tbl * tbl).sum(-1, keepdims=True))
    tbl = tbl * np.minimum(1.0, 1.0 / (nrm + 1e-7))
    W1 = np.asarray(d['early_W1'], f32)
    T1 = (tbl @ W1[:EMB]).astype(f32)                        # [OPS, 256]
    inv_std = (1.0 / (np.asarray(d['node_feat_std'], f32) + 1e-4)).astype(f32)
    w1b = (W1[EMB:] * inv_std[:, None]).astype(f32)          # [OPF, 256]
    b0 = (-(np.asarray(d['node_feat_mean'], f32) * inv_std) @ W1[EMB:]).astype(f32)

    perms, nAs = [], []
    rho = np.arange(HID)
    wl_l, wr_l, bl_l, br_l, gb_l, ratt_l = [], [], [], [], [], []
    for i in range(4):
        att = np.asarray(d['gat_att'][i], f32)
        pos_c = np.where(att >= 0)[0]
        neg_c = np.where(att < 0)[0]
        perm = np.concatenate([pos_c, neg_c])
        nAs.append(len(pos_c))
        assert np.abs(att[perm]).min() > 1e-12
        wl_l.append((np.asarray(d['gat_Wl'][i], f32)[rho][:, perm]
                     * att[perm][None, :]).astype(f32))
        wr_l.append((np.asarray(d['gat_Wr'][i], f32)[rho][:, perm]
                     * att[perm][None, :]).astype(f32))
        bl_l.append(((np.asarray(d['gat_bl'][i], f32) * att)[perm]).astype(f32))
        br_l.append(((np.asarray(d['gat_br'][i], f32) * att)[perm]).astype(f32))
        gb_l.append(np.asarray(d['gat_bias'][i], f32)[perm].astype(f32))
        ratt_l.append((1.0 / att[perm]).astype(f32))
        perms.append(perm)
        rho = perm

    cf_inv = (1.0 / (np.asarray(d['config_feat_std'], f32) + 1e-4)).astype(f32)
    LW1 = np.asarray(d['late_W1'], f32)
    w1c = (LW1[:CF] * cf_inv[:, None]).astype(f32)
    bc0 = (-(np.asarray(d['config_feat_mean'], f32) * cf_inv) @ LW1[:CF]).astype(f32)
    w1p = LW1[CF:][perms[3]].astype(f32)

    # ---- node balancing + edge sharding (self loops handled separately) ----
    NS = N_IN // NCORES
    NP_ = ((NS + 127) // 128) * 128
    NDT = NP_ // 128
    ei = np.asarray(d['edge_index']).astype(np.int64)
    src, dst = ei[0], ei[1]
    owner = dst // NS
    loc = dst % NS

    pos_all = []
    max_load = 0
    for k in range(NCORES):
        deg = np.bincount(loc[owner == k], minlength=NS)
        pos_k, ml = _balance(deg, NS, NDT)
        pos_all.append(pos_k)
        max_load = max(max_load, ml)
    tpd = max(1, (max_load + 127) // 128)
    cfg = Cfg(NS, NP_, tpd, C_IN, ((C_IN + 127) // 128) * 128, nAs)
    L = cfg.L

    src_owner = src // NS
    src_pad = np.empty_like(src)
    for k in range(NCORES):
        m = src_owner == k
        src_pad[m] = k * NP_ + pos_all[k][src[m] % NS]

    per_core = []
    for k in range(NCORES):
        m = owner == k
        sk = src_pad[m]
        newloc = pos_all[k][loc[m]]
        grp = newloc // 128
        rel = newloc % 128
        src_g = np.zeros(L, np.int64)
        dst_rel = np.full(L, -1000.0, f32)
        for g in range(NDT):
            gm = grp == g
            c = int(gm.sum())
            assert c <= tpd * 128
            base = g * tpd * 128
            src_g[base:base + c] = sk[gm]
            dst_rel[base:base + c] = rel[gm].astype(f32)
        per_core.append((src_g, dst_rel))

    # ---- per-core input maps ----
    bf16 = np.dtype('bfloat16') if hasattr(np, 'bfloat16') else None
    import ml_dtypes
    bf16 = ml_dtypes.bfloat16

    nf = np.asarray(d['node_feat'], f32)
    opc = np.asarray(d['node_opcode']).astype(np.int64)
    cfp = np.zeros((cfg.CP, CF), f32)
    cfp[:C_IN] = np.asarray(d['config_feat'], f32)

    NSV = 18
    sv = np.zeros((HID, NSV), f32)
    sv[:, 0] = b0
    for i in range(4):
        sv[:, 3 + 4 * i] = gb_l[i]
        sv[:, 4 + 4 * i] = ratt_l[i]
    sv[:, 17] = bc0

    w2 = np.asarray(d['early_W2'], f32)
    w2l = np.asarray(d['late_W2'], f32)

    def bf(x):
        return np.asarray(x, f32).astype(bf16)

    shared = {
        't1': bf(T1),
        'w1ba': bf(w1b[:128]), 'w1bb': bf(w1b[128:]),
        'w2a': bf(w2[:128]), 'w2b': bf(w2[128:]),
        'w1c': bf(w1c),
        'w1pa': bf(w1p[:128]), 'w1pb': bf(w1p[128:]),
        'w2la': bf(w2l[:128]), 'w2lb': bf(w2l[128:]),
        'predw': bf(np.asarray(d['pred_W'], f32)),
        'predb': np.asarray(d['pred_b'], f32).reshape(1, 1),
        'sv_lo': sv[:128].copy(), 'sv_hi': sv[128:].copy(),
        'cf': cfp,
    }
    for i in range(4):
        # fused [Wl | Wr] so xl and xr come from one N=512 matmul set
        shared[f'wlr{i}a'] = bf(np.concatenate([wl_l[i][:128], wr_l[i][:128]], 1))
        shared[f'wlr{i}b'] = bf(np.concatenate([wl_l[i][128:], wr_l[i][128:]], 1))
        shared[f'blrrow{i}'] = bf(np.concatenate([bl_l[i], br_l[i]]).reshape(1, 2 * HID))
    sel = np.zeros((16, 16 * 128), f32)
    for j in range(16):
        sel[j, j * 128:(j + 1) * 128] = 1.0
    shared['selc'] = bf(sel)

    in_maps = []
    for k in range(NCORES):
        src_g, dst_rel = per_core[k]
        nfk = np.zeros((cfg.NP, OPF), f32)
        nfk[pos_all[k]] = nf[k * NS:(k + 1) * NS]
        ok = np.zeros(cfg.NP, np.int64)
        ok[pos_all[k]] = opc[k * NS:(k + 1) * NS]
        m = dict(shared)
        m['nf'] = nfk
        m['opcol'] = ok.reshape(NDT, 128).astype(f32).T.copy()
        m['drelcols'] = dst_rel.reshape(cfg.TALL, 128).T.copy()
        m['srcidx'] = _wrap_idx(src_g)
        in_maps.append(m)
    return cfg, in_maps


# ---------------------------------------------------------------------------
# program builder
# ---------------------------------------------------------------------------

def build_program(cfg: Cfg):
    nc = bacc.Bacc("TRN2", target_bir_lowering=False, debug=False,
                   num_devices=NCORES, num_swdge_queues=2)
    NP_, NS, TPD, NDT, L = cfg.NP, cfg.NS, cfg.TPD, cfg.NDT, cfg.L
    NT = NDT
    TALL = cfg.TALL
    REPL = [list(range(NCORES))]

    def din(name, shape, dt=F32):
        return nc.dram_tensor(name, list(shape), dt, kind="ExternalInput")

    # ---- external inputs ----
    nf_d = din('nf', (NP_, OPF))
    t1_d = din('t1', (OPS, HID), GDT)
    opcol_d = din('opcol', (128, NDT))
    drelcols_d = din('drelcols', (128, TALL))
    srcidx_d = din('srcidx', (128, L // 16), I16)
    w1ba_d = din('w1ba', (128, HID), GDT)
    w1bb_d = din('w1bb', (OPF - 128, HID), GDT)
    w2_d = [din('w2a', (128, HID), GDT), din('w2b', (128, HID), GDT)]
    wlr_d = [[din(f'wlr{i}a', (128, 2 * HID), GDT),
              din(f'wlr{i}b', (128, 2 * HID), GDT)] for i in range(4)]
    blr_d = [din(f'blrrow{i}', (1, 2 * HID), GDT) for i in range(4)]
    selc_d = din('selc', (16, 16 * 128), GDT)
    w1c_d = din('w1c', (CF, HID), GDT)
    w1p_d = [din('w1pa', (128, HID), GDT), din('w1pb', (128, HID), GDT)]
    w2l_d = [din('w2la', (128, 128), GDT), din('w2lb', (128, 128), GDT)]
    predw_d = din('predw', (128, 1), GDT)
    predb_d = din('predb', (1, 1))
    sv_d = [din('sv_lo', (128, 18)), din('sv_hi', (128, 18))]
    cf_d = din('cf', (cfg.CP, CF))
    out_d = nc.dram_tensor('out', [1, cfg.CP], F32, kind="ExternalOutput")

    # ---- internal DRAM ----
    xl_own = [nc.dram_tensor(f'xl_own{i}', [NP_, HID], GDT) for i in range(4)]
    xl_full = [nc.dram_tensor(f'xl_full{i}', [cfg.NPG, HID], GDT,
                              addr_space="Shared") for i in range(4)]
    ar_in = [nc.dram_tensor(f'ar_in{i}', [128, 4], F32) for i in range(6)]
    ar_out = [nc.dram_tensor(f'ar_out{i}', [128 * NCORES, 4], F32,
                             addr_space="Shared") for i in range(6)]
    pool_in = nc.dram_tensor('pool_in', [128, 4], F32)
    pool_out = nc.dram_tensor('pool_out', [128 * NCORES, 4], F32,
                              addr_space="Shared")

    with tile.TileContext(nc) as tc, __import__('contextlib').ExitStack() as ctx:
        const = ctx.enter_context(tc.tile_pool(name="const", bufs=1))
        big = ctx.enter_context(tc.tile_pool(name="big", bufs=1))
        work = ctx.enter_context(tc.tile_pool(name="work", bufs=3))
        col = ctx.enter_context(tc.tile_pool(name="col", bufs=6))

        # ------ constants ------
        ident = const.tile([128, 128], F32, tag="ident", name="ident")
        make_identity(nc, ident[:])
        ident_bf = const.tile([128, 128], GDT, tag="identbf", name="identbf")
        nc.vector.tensor_copy(ident_bf[:], ident[:])
        iota_i = const.tile([128, 128], I32, tag="iota_i", name="iota_i")
        nc.gpsimd.iota(iota_i[:], pattern=[[1, 128]], base=0, channel_multiplier=0)
        iota_f = const.tile([128, 128], F32, tag="iota_f", name="iota_f")
        nc.vector.tensor_copy(iota_f[:], iota_i[:])
        ones_row = const.tile([1, 512], GDT, tag="onesrow", name="onesrow")
        nc.gpsimd.memset(ones_row[:], 1.0)
        ones_bf = const.tile([128, 1], GDT, tag="onesbf", name="onesbf")
        nc.gpsimd.memset(ones_bf[:], 1.0)
        zero_col = const.tile([128, 1], F32, tag="zeroc", name="zeroc")
        nc.gpsimd.memset(zero_col[:], 0.0)
        nc.const_aps.aps[(F32, 0.0)] = zero_col[:]
        eps_col = const.tile([128, 1], F32, tag="epsc", name="epsc")
        nc.gpsimd.memset(eps_col[:], 1e-5)

        def load_const(dram, tag):
            t = const.tile(list(dram.shape), dram.dtype, tag=tag)
            nc.sync.dma_start(out=t[:], in_=dram[:])
            return t

        srcidx = load_const(srcidx_d, 'srcidx')
        opcol = load_const(opcol_d, 'opcol')
        drelcols = load_const(drelcols_d, 'drelcols')
        t1 = load_const(t1_d, 't1')
        w1ba = load_const(w1ba_d, 'w1ba')
        w1bb = load_const(w1bb_d, 'w1bb')
        w2 = [load_const(w2_d[j], f'w2{j}') for j in range(2)]
        wlr = [[load_const(wlr_d[i][j], f'wlr{i}{j}') for j in range(2)]
               for i in range(4)]
        blr = [load_const(blr_d[i], f'blr{i}') for i in range(4)]
        selc = load_const(selc_d, 'selc')
        w1c = load_const(w1c_d, 'w1c')
        w1p = [load_const(w1p_d[j], f'w1p{j}') for j in range(2)]
        w2l = [load_const(w2l_d[j], f'w2l{j}') for j in range(2)]
        predw = load_const(predw_d, 'predw')
        predb = load_const(predb_d, 'predb')
        sv = [load_const(sv_d[j], f'sv{j}') for j in range(2)]

        # ------ persistent big tiles ------
        raw = [big.tile([128, NP_], F32, tag=f"raw{m}", name=f"raw{m}") for m in range(2)]
        xt = [big.tile([128, NP_], GDT, tag=f"x{m}", name=f"x{m}") for m in range(2)]
        xl_sb = big.tile([128, NDT * HID], GDT, tag="xlsb", name="xlsb")
        xr_sb = big.tile([128, NDT * HID], GDT, tag="xrsb", name="xrsb")
        oh01 = big.tile([128, TALL * 128], F8, tag="oh01", name="oh01")
        ohT = big.tile([128, TALL * 128], F8, tag="ohT", name="ohT")

        blocks = [(s, min(s + 512, NP_)) for s in range(0, NP_, 512)]

        def stats_tiles(tagp):
            return [work.tile([128, max(len(blocks), NDT)], F32,
                              tag=f"st{m}", name=f"{tagp}{m}") for m in range(2)]

        sqscr = big.tile([128, 512], F32, tag="sqscr", name="sqscr")

        # --- evac: dst[:, c0:c1] = src*scale + bias, stats over cols < NS ---
        def evac_cm(dst, src_ap, c0, c1, mc, st1, st2, bi, bias_col, scale_col,
                    nreal=NS):
            def one(a, b, accum):
                kw = {'accum_out': st1[mc][:, bi:bi + 1]} if accum else {}
                nc.scalar.activation(dst[:, a:b], src_ap[:, a - c0:b - c0],
                                     AF.Identity, bias=bias_col, scale=scale_col,
                                     **kw)
                if accum:
                    nc.vector.scalar_tensor_tensor(
                        sqscr[:, :b - a], dst[:, a:b], 1.0, dst[:, a:b],
                        ALU.mult, ALU.mult,
                        accum_out=st2[mc][:, bi:bi + 1])
            if c0 >= nreal:
                one(c0, c1, False)
            elif c1 <= nreal:
                one(c0, c1, True)
            else:
                one(c0, nreal, True)
                one(nreal, c1, False)

        # --- stats exchange via tiny AllGather + local reduce ---
        def stats_allgather(st1, st2, ar_i, ar_o, ntotal, nblk):
            art = work.tile([128, 4], F32, tag="art", name="art")
            for m in range(2):
                nc.vector.tensor_reduce(art[:, 2 * m:2 * m + 1], st1[m][:, :nblk],
                                        AX.X, ALU.add)
                nc.vector.tensor_reduce(art[:, 2 * m + 1:2 * m + 2], st2[m][:, :nblk],
                                        AX.X, ALU.add)
            nc.sync.dma_start(out=ar_i[:], in_=art[:])
            nc.gpsimd.collective_compute(
                "AllGather", ALU.bypass, replica_groups=REPL,
                ins=[ar_i[:]], outs=[ar_o[:]])
            arr = work.tile([128, NCORES * 4], F32, tag="arrg", name="arrg")
            nc.sync.dma_start(out=arr[:].rearrange("p (k v) -> p k v", v=4),
                              in_=ar_o[:].rearrange("(k p) v -> p k v", p=128))
            arr3 = arr[:].rearrange("p (k v) -> p k v", v=4)
            rs_l, nmr_l = [], []
            for m in range(2):
                s1 = col.tile([128, 1], F32, tag="cs1", name="cs1")
                s2 = col.tile([128, 1], F32, tag="cs2", name="cs2")
                nc.vector.tensor_reduce(s1[:], arr3[:, :, 2 * m], AX.X, ALU.add)
                nc.vector.tensor_reduce(s2[:], arr3[:, :, 2 * m + 1], AX.X, ALU.add)
                rs, nmr = _finish_stats(s1, s2, ntotal)
                rs_l.append(rs)
                nmr_l.append(nmr)
            return rs_l, nmr_l

        def _finish_stats(s1, s2, ntotal):
            mu = col.tile([128, 1], F32, tag="mu", name="mu")
            nc.vector.tensor_scalar(mu[:], s1[:], 1.0 / ntotal, None, ALU.mult)
            mu2 = col.tile([128, 1], F32, tag="mu2", name="mu2")
            nc.scalar.activation(mu2[:], mu[:], AF.Square)
            var = col.tile([128, 1], F32, tag="var", name="var")
            nc.vector.scalar_tensor_tensor(var[:], s2[:], 1.0 / ntotal, mu2[:],
                                           ALU.mult, ALU.subtract)
            sd = col.tile([128, 1], F32, tag="sd", name="sd")
            nc.scalar.activation(sd[:], var[:], AF.Sqrt, bias=eps_col[:])
            rs = col.tile([128, 1], F32, tag="rs", name="rs")
            nc.vector.reciprocal(rs[:], sd[:])
            t = col.tile([128, 1], F32, tag="nmrt", name="nmrt")
            nc.vector.tensor_tensor(t[:], mu[:], rs[:], ALU.mult)
            nmr = col.tile([128, 1], F32, tag="nmr", name="nmr")
            nc.vector.tensor_scalar(nmr[:], t[:], -1.0, None, ALU.mult)
            return rs, nmr

        def norm_gelu(src_tiles, dst_tiles, rs_l, nmr_l):
            for m in range(2):
                nc.scalar.activation(dst_tiles[m][:], src_tiles[m][:], AF.Gelu,
                                     bias=nmr_l[m][:], scale=rs_l[m][:])

        # =================== early stage ===================
        early = tc.alloc_tile_pool(name="early", bufs=1)
        ps_early = tc.alloc_tile_pool(name="pse", bufs=2, space="PSUM")
        nfTa = early.tile([128, NP_], GDT, tag="nfTa", name="nfTa")
        nfTb = early.tile([OPF - 128, NP_], GDT, tag="nfTb", name="nfTb")
        ohop = early.tile([OPS, NP_], GDT, tag="ohop", name="ohop")
        nf_nm = early.tile([128, NT * OPF], F32, tag="nf_nm", name="nf_nm")
        nc.sync.dma_start(
            out=nf_nm[:].rearrange("p (t c) -> p t c", c=OPF),
            in_=nf_d[:].rearrange("(t p) c -> p t c", p=128))

        for t in range(NT):
            ps = ps_early.tile([128, 512], F32, tag="mm", name="etr")
            nc.tensor.matmul(ps[:, :128], lhsT=nf_nm[:, t * OPF:t * OPF + 128],
                             rhs=ident[:], is_transpose=True, start=True, stop=True)
            nc.vector.tensor_copy(nfTa[:, t * 128:(t + 1) * 128], ps[:, :128])
            ps2 = ps_early.tile([128, 512], F32, tag="mm", name="etr2")
            nc.tensor.matmul(ps2[:OPF - 128, :128],
                             lhsT=nf_nm[:, t * OPF + 128:(t + 1) * OPF],
                             rhs=ident[:], is_transpose=True, start=True, stop=True)
            nc.vector.tensor_copy(nfTb[:, t * 128:(t + 1) * 128],
                                  ps2[:OPF - 128, :128])
            # opcode one-hot (op on partitions): ohop[op, n] = (opc[n] == op)
            opsb = work.tile([128, OPS], F32, tag="opsb", name="opsb")
            nc.scalar.activation(opsb[:], iota_f[:, :OPS], AF.Identity,
                                 bias=opcol[:, t:t + 1], scale=0.0)
            ohnf = work.tile([128, OPS], F32, tag="ohnf", name="ohnf")
            nc.vector.tensor_tensor(ohnf[:], opsb[:], iota_f[:, :OPS],
                                    ALU.is_equal)
            ps3 = ps_early.tile([128, 512], F32, tag="mm", name="eoh")
            nc.tensor.matmul(ps3[:OPS, :128], lhsT=ohnf[:], rhs=ident[:],
                             is_transpose=True, start=True, stop=True)
            nc.vector.tensor_copy(ohop[:, t * 128:(t + 1) * 128], ps3[:OPS, :128])

        # early layer 1: raw = nf @ w1b + T1[op] + b0   (channel-major)
        st1 = stats_tiles("e1s1")
        st2 = stats_tiles("e1s2")
        for mc in range(2):
            for bi, (s, e) in enumerate(blocks):
                w = e - s
                ps = ps_early.tile([128, 512], F32, tag="mm", name="mm")
                nc.tensor.matmul(ps[:, :w], lhsT=w1ba[:, mc * 128:(mc + 1) * 128],
                                 rhs=nfTa[:, s:e], start=True, stop=False)
                nc.tensor.matmul(ps[:, :w], lhsT=w1bb[:, mc * 128:(mc + 1) * 128],
                                 rhs=nfTb[:, s:e], start=False, stop=False)
                nc.tensor.matmul(ps[:, :w], lhsT=t1[:, mc * 128:(mc + 1) * 128],
                                 rhs=ohop[:, s:e], start=False, stop=True)
                evac_cm(raw[mc], ps[:, :w], s, e, mc, st1, st2, bi,
                        sv[mc][:, 0:1], 1.0)
        rs_l, nmr_l = stats_allgather(st1, st2, ar_in[0], ar_out[0], cfg.N,
                                      len(blocks))
        norm_gelu(raw, xt, rs_l, nmr_l)

        # early layer 2: raw = x @ w2
        st1 = stats_tiles("e2s1")
        st2 = stats_tiles("e2s2")
        for mc in range(2):
            for bi, (s, e) in enumerate(blocks):
                w = e - s
                ps = ps_early.tile([128, 512], F32, tag="mm", name="mm")
                for kc in range(2):
                    nc.tensor.matmul(ps[:, :w], lhsT=w2[kc][:, mc * 128:(mc + 1) * 128],
                                     rhs=xt[kc][:, s:e], start=(kc == 0),
                                     stop=(kc == 1))
                evac_cm(raw[mc], ps[:, :w], s, e, mc, st1, st2, bi,
                        zero_col[:], 1.0)
        rs_l, nmr_l = stats_allgather(st1, st2, ar_in[1], ar_out[1], cfg.N,
                                      len(blocks))
        norm_gelu(raw, xt, rs_l, nmr_l)
        early.release()
        ps_early.release()

        # =================== static one-hots (all layers) ===================
        ps_oh = tc.alloc_tile_pool(name="psoh", bufs=2, space="PSUM")
        for t in range(TALL):
            dsb = work.tile([128, 128], F32, tag="dsb", name="dsb")
            nc.scalar.activation(dsb[:], iota_f[:], AF.Identity,
                                 bias=drelcols[:, t:t + 1], scale=0.0)
            ohf = work.tile([128, 128], F32, tag="ohf", name="ohf")
            nc.vector.tensor_tensor(ohf[:], dsb[:], iota_f[:], ALU.is_equal)
            nc.vector.tensor_copy(oh01[:, t * 128:(t + 1) * 128], ohf[:])
            psr = ps_oh.tile([128, 128], F32, tag="dc", name="dr")
            nc.tensor.matmul(psr[:], lhsT=ohf[:], rhs=ident[:],
                             is_transpose=True, start=True, stop=True)
            nc.vector.tensor_copy(ohT[:, t * 128:(t + 1) * 128], psr[:])
        ps_oh.release()

        # =================== GAT layers ===================
        gat = tc.alloc_tile_pool(name="gath", bufs=2)
        ps_ht = tc.alloc_tile_pool(name="psht", bufs=2, space="PSUM")
        ps_agg = tc.alloc_tile_pool(name="psagg", bufs=2, space="PSUM")
        ps_sm = tc.alloc_tile_pool(name="pssm", bufs=2, space="PSUM")
        SGT = SGN * TPD                      # gathered tiles per supergroup
        NSG = (NDT + SGN - 1) // SGN

        for li in range(4):
            nA = cfg.nA[li]
            # ---- fused xl|xr node-major (bias folded in as K=1 matmul) ----
            for t in range(NT):
                ps = ps_sm.tile([128, 512], F32, tag="nm", name="nm")
                for kc in range(2):
                    nc.tensor.matmul(
                        ps[:], lhsT=xt[kc][:, t * 128:(t + 1) * 128],
                        rhs=wlr[li][kc][:], start=(kc == 0), stop=False)
                nc.tensor.matmul(ps[:], lhsT=ones_row[:1, :128],
                                 rhs=blr[li][:1, :], start=False, stop=True)
                nc.vector.tensor_copy(xl_sb[:, t * HID:(t + 1) * HID], ps[:, :HID])
                nc.vector.tensor_copy(xr_sb[:, t * HID:(t + 1) * HID], ps[:, HID:])
            nc.sync.dma_start(
                out=xl_own[li][:].rearrange("(t p) c -> p t c", p=128),
                in_=xl_sb[:].rearrange("p (t c) -> p t c", c=HID))
            nc.gpsimd.collective_compute(
                "AllGather", ALU.bypass, replica_groups=REPL,
                ins=[xl_own[li][:]], outs=[xl_full[li][:]])

            # ---- edge processing (agg phase pipelined one group behind) ----
            st1 = stats_tiles("gs1")
            st2 = stats_tiles("gs2")
            TT = TPD + 1                      # gathered + self

            def score_phase(g, xlg, base_t):
                xl_g = xl_sb[:, g * HID:(g + 1) * HID]
                xr_g = xr_sb[:, g * HID:(g + 1) * HID]
                ecols = col.tile([128, 16], F32, tag="ecols", name="ecols")
                for h0 in range(0, TT, HB):
                    hbn = min(HB, TT - h0)
                    htp = ps_ht.tile([128, HB * HID], F32, tag="ht", name="ht")
                    for jj in range(hbn):
                        j = h0 + jj
                        sl = htp[:, jj * HID:(jj + 1) * HID]
                        if j < TPD:
                            tix = g * TPD + j
                            nc.tensor.matmul(
                                sl, lhsT=ohT[:, tix * 128:(tix + 1) * 128],
                                rhs=xr_g, start=True, stop=False)
                            nc.tensor.matmul(
                                sl, lhsT=ident_bf[:],
                                rhs=xlg[:, (base_t + j) * HID:(base_t + j + 1) * HID],
                                start=False, stop=True)
                        else:
                            nc.tensor.matmul(sl, lhsT=ident_bf[:], rhs=xr_g,
                                             start=True, stop=False)
                            nc.tensor.matmul(sl, lhsT=ident_bf[:], rhs=xl_g,
                                             start=False, stop=True)
                    scr = work.tile([128, HB * HID], GDT, tag="scr", name="scr")
                    h3 = htp[:, :hbn * HID].rearrange("p (t c) -> p t c", c=HID)
                    s3 = scr[:, :hbn * HID].rearrange("p (t c) -> p t c", c=HID)
                    nc.scalar.activation(s3[:, :, :nA], h3[:, :, :nA], AF.Lrelu,
                                         alpha=0.2)
                    nc.scalar.activation(s3[:, :, nA:], h3[:, :, nA:], AF.Lrelu,
                                         scale=-1.0, alpha=0.2)
                    ra = col.tile([128, HB], F32, tag="ra", name="ra")
                    rb = col.tile([128, HB], F32, tag="rb", name="rb")
                    nc.vector.tensor_reduce(ra[:, :hbn], s3[:, :, :nA], AX.X,
                                            ALU.add)
                    nc.vector.tensor_reduce(rb[:, :hbn], s3[:, :, nA:], AX.X,
                                            ALU.add)
                    nc.vector.tensor_tensor(ecols[:, h0:h0 + hbn], ra[:, :hbn],
                                            rb[:, :hbn], ALU.subtract)
                wcols = col.tile([128, 16], F32, tag="wcols", name="wcols")
                nc.scalar.activation(wcols[:, :TT], ecols[:, :TT], AF.Exp)
                pswt = ps_sm.tile([128, 512], F32, tag="nm", name="wt")
                nc.tensor.matmul(pswt[:16, :128], lhsT=wcols[:, :16], rhs=ident[:],
                                 is_transpose=True, start=True, stop=True)
                wrows = work.tile([16, 128], GDT, tag="wrows", name="wrows")
                nc.vector.tensor_copy(wrows[:], pswt[:16, :128])
                return wcols, wrows

            def agg_phase(g, xlg, base_t, wcols, wrows):
                xl_g = xl_sb[:, g * HID:(g + 1) * HID]
                agg = ps_agg.tile([128, 512], F32, tag="agg", name="agg")
                ps256 = agg[:, :HID]
                ps1 = agg[:, HID:HID + 1]
                # batched w-broadcasts -> one DVE multiply per 4 tiles
                ohsall = work.tile([128, TPD * 128], GDT, tag="ohsall",
                                   name="ohsall")
                for c0 in range(0, TPD, 4):
                    cn = min(4, TPD - c0)
                    wall = ps_sm.tile([128, 512], F32, tag="wc4", name="wall")
                    for jj in range(cn):
                        j = c0 + jj
                        nc.tensor.matmul(wall[:, jj * 128:(jj + 1) * 128],
                                         lhsT=wrows[:],
                                         rhs=selc[:, j * 128:(j + 1) * 128],
                                         start=True, stop=True)
                    nc.vector.tensor_tensor(
                        ohsall[:, c0 * 128:(c0 + cn) * 128],
                        oh01[:, (g * TPD + c0) * 128:(g * TPD + c0 + cn) * 128],
                        wall[:, :cn * 128], ALU.mult)
                for j in range(TPD):
                    nc.tensor.matmul(ps256, lhsT=ohsall[:, j * 128:(j + 1) * 128],
                                     rhs=xlg[:, (base_t + j) * HID:(base_t + j + 1) * HID],
                                     start=(j == 0), stop=(j == TPD - 1))
                    nc.tensor.matmul(ps1, lhsT=ohsall[:, j * 128:(j + 1) * 128],
                                     rhs=ones_bf[:],
                                     start=(j == 0), stop=(j == TPD - 1))
                # ---- evacuation ----
                wself = wcols[:, TPD:TPD + 1]
                dcol = col.tile([128, 1], F32, tag="dcol", name="dcol")
                nc.vector.scalar_tensor_tensor(dcol[:], ps1, 1e-16, wself,
                                               ALU.add, ALU.add)
                rcol = col.tile([128, 1], F32, tag="rcol", name="rcol")
                nc.vector.reciprocal(rcol[:], dcol[:])
                smsg = work.tile([128, HID], F32, tag="smsg", name="smsg")
                nc.scalar.activation(smsg[:], xl_g, AF.Copy, scale=wself)
                msum = work.tile([128, HID], F32, tag="msum", name="msum")
                nc.vector.scalar_tensor_tensor(msum[:], ps256, 1.0, smsg[:],
                                               ALU.mult, ALU.add)
                msg = work.tile([128, HID], F32, tag="msg", name="msg")
                nc.scalar.activation(msg[:], msum[:], AF.Copy, scale=rcol[:])
                for mc in range(2):
                    pstr = ps_sm.tile([128, 512], F32, tag="nm", name="tr")
                    nc.tensor.matmul(pstr[:, :128],
                                     lhsT=msg[:, mc * 128:(mc + 1) * 128],
                                     rhs=ident[:], is_transpose=True,
                                     start=True, stop=True)
                    evac_cm(raw[mc], pstr[:, :128], g * 128, (g + 1) * 128, mc,
                            st1, st2, g, sv[mc][:, 3 + 4 * li:4 + 4 * li],
                            sv[mc][:, 4 + 4 * li:5 + 4 * li])

            xlg = None
            qn = [0]
            prev = None
            for g in range(NDT):
                if g % SGN == 0:
                    sg = g // SGN
                    ntiles = min(SGT, TALL - sg * SGT)
                    xlg = gat.tile([128, SGT * HID], GDT, tag="xlg", name="xlg")
                    total = ntiles * 128
                    i0 = sg * SGT * 128
                    done = 0
                    x3 = xlg[:].rearrange("p (t c) -> p t c", c=HID)
                    while done < total:
                        n = min(CHUNK, total - done)
                        nc.gpsimd.dma_gather(
                            out_ap=x3[:, done // 128:(done + n) // 128, :],
                            in_ap=xl_full[li][:],
                            idxs_ap=srcidx[:, (i0 + done) // 16:(i0 + done + n) // 16],
                            num_idxs=n, num_idxs_reg=n, elem_size=HID,
                            queue_num=qn[0] % 2)
                        qn[0] += 1
                        done += n
                base_t = (g % SGN) * TPD
                wc, wr = score_phase(g, xlg, base_t)
                if prev is not None:
                    agg_phase(*prev)
                prev = (g, xlg, base_t, wc, wr)
            agg_phase(*prev)
            rs_l, nmr_l = stats_allgather(st1, st2, ar_in[2 + li], ar_out[2 + li],
                                          cfg.N, NDT)
            norm_gelu(raw, xt, rs_l, nmr_l)

        ps_sm.release()
        ps_agg.release()
        ps_ht.release()
        gat.release()

        # =================== pooling ===================
        pt = work.tile([128, 4], F32, tag="pt", name="pt")
        for m in range(2):
            nc.vector.tensor_reduce(pt[:, m:m + 1], xt[m][:, :NS], AX.X, ALU.add)
            nc.vector.tensor_reduce(pt[:, 2 + m:3 + m], xt[m][:, :NS], AX.X, ALU.max)
        nc.sync.dma_start(out=pool_in[:], in_=pt[:])
        nc.gpsimd.collective_compute(
            "AllGather", ALU.bypass, replica_groups=REPL,
            ins=[pool_in[:]], outs=[pool_out[:]])
        pg = work.tile([128, NCORES * 4], F32, tag="pg", name="pg")
        nc.sync.dma_start(out=pg[:].rearrange("p (k v) -> p k v", v=4),
                          in_=pool_out[:].rearrange("(k p) v -> p k v", p=128))
        pg3 = pg[:].rearrange("p (k v) -> p k v", v=4)
        pool_c = []
        for m in range(2):
            s_ = col.tile([128, 1], F32, tag="psum_c", name="psum_c")
            nc.vector.tensor_reduce(s_[:], pg3[:, :, m], AX.X, ALU.add)
            mx = col.tile([128, 1], F32, tag="pmax_c", name="pmax_c")
            nc.vector.tensor_reduce(mx[:], pg3[:, :, 2 + m], AX.X, ALU.max)
            pc = col.tile([128, 1], GDT, tag="pool_c", name="pool_c")
            pcf = col.tile([128, 1], F32, tag="pool_cf", name="pool_cf")
            nc.vector.scalar_tensor_tensor(pcf[:], s_[:], 1.0 / cfg.N, mx[:],
                                           ALU.mult, ALU.add)
            nc.vector.tensor_copy(pc[:], pcf[:])
            pool_c.append(pc)

        # =================== late MLP (replicated) ===================
        cblocks = [(s, min(s + 512, cfg.CP)) for s in range(0, cfg.CP, 512)]
        NCT = cfg.CP // 128
        late = tc.alloc_tile_pool(name="late", bufs=1)
        ps_late = tc.alloc_tile_pool(name="psl", bufs=2, space="PSUM")
        cf_nm = work.tile([128, NCT * CF], F32, tag="cf_nm", name="cf_nm")
        nc.sync.dma_start(out=cf_nm[:].rearrange("p (t c) -> p t c", c=CF),
                          in_=cf_d[:].rearrange("(t p) c -> p t c", p=128))
        cfT = late.tile([CF, cfg.CP], GDT, tag="cfT", name="cfT")
        for t in range(NCT):
            ps = ps_late.tile([128, 512], F32, tag="mm", name="ctr")
            nc.tensor.matmul(ps[:CF, :128], lhsT=cf_nm[:, t * CF:(t + 1) * CF],
                             rhs=ident[:], is_transpose=True, start=True, stop=True)
            nc.vector.tensor_copy(cfT[:, t * 128:(t + 1) * 128], ps[:CF, :128])

        # vec1 = w1p^T @ pool  (+ bc0)
        vcol = []
        for mc in range(2):
            ps = ps_late.tile([128, 512], F32, tag="v1", name="v1")
            for kc in range(2):
                nc.tensor.matmul(ps[:, :1], lhsT=w1p[kc][:, mc * 128:(mc + 1) * 128],
                                 rhs=pool_c[kc][:], start=(kc == 0), stop=(kc == 1))
            v = col.tile([128, 1], F32, tag="vcol", name="vcol")
            nc.vector.scalar_tensor_tensor(v[:], sv[mc][:, 17:18], 1.0, ps[:, :1],
                                           ALU.mult, ALU.add)
            vcol.append(v)

        h1r = [late.tile([128, cfg.CP], F32, tag=f"h1r{m}", name=f"h1r{m}")
               for m in range(2)]
        h1 = [late.tile([128, cfg.CP], GDT, tag=f"h1_{m}", name=f"h1_{m}")
              for m in range(2)]
        h2r = late.tile([128, cfg.CP], F32, tag="h2r", name="h2r")
        h2 = late.tile([128, cfg.CP], GDT, tag="h2_", name="h2_")

        def late_stats_norm(src_tiles, dst_tiles, st1, st2, nblk, nmc):
            for m in range(nmc):
                s1 = col.tile([128, 1], F32, tag="cs1", name="cs1")
                s2 = col.tile([128, 1], F32, tag="cs2", name="cs2")
                nc.vector.tensor_reduce(s1[:], st1[m][:, :nblk], AX.X, ALU.add)
                nc.vector.tensor_reduce(s2[:], st2[m][:, :nblk], AX.X, ALU.add)
                rs, nmr = _finish_stats(s1, s2, cfg.C)
                nc.scalar.activation(dst_tiles[m][:], src_tiles[m][:], AF.Gelu,
                                     bias=nmr[:], scale=rs[:])

        # h1 = gelu(cfgnorm(cf @ w1c + vec1))
        st1 = stats_tiles("l1s1")
        st2 = stats_tiles("l1s2")
        for mc in range(2):
            for bi, (s, e) in enumerate(cblocks):
                w = e - s
                ps = ps_late.tile([128, 512], F32, tag="mm", name="mm")
                nc.tensor.matmul(ps[:, :w], lhsT=w1c[:, mc * 128:(mc + 1) * 128],
                                 rhs=cfT[:, s:e], start=True, stop=True)
                evac_cm(h1r[mc], ps[:, :w], s, e, mc, st1, st2, bi,
                        vcol[mc][:], 1.0, nreal=cfg.C)
        late_stats_norm(h1r, h1, st1, st2, len(cblocks), 2)

        # h2 = gelu(cfgnorm(h1 @ w2l))   (128 out channels -> mc=0 only)
        st1 = stats_tiles("l2s1")
        st2 = stats_tiles("l2s2")
        for bi, (s, e) in enumerate(cblocks):
            w = e - s
            ps = ps_late.tile([128, 512], F32, tag="mm", name="mm")
            for kc in range(2):
                nc.tensor.matmul(ps[:, :w], lhsT=w2l[kc][:], rhs=h1[kc][:, s:e],
                                 start=(kc == 0), stop=(kc == 1))
            evac_cm(h2r, ps[:, :w], s, e, 0, st1, st2, bi, zero_col[:], 1.0,
                    nreal=cfg.C)
        late_stats_norm([h2r], [h2], st1, st2, len(cblocks), 1)

        # pred: out = h2^T @ predw + predb
        outsb = work.tile([1, cfg.CP], F32, tag="outsb", name="outsb")
        for (s, e) in cblocks:
            w = e - s
            ps = ps_late.tile([128, 512], F32, tag="mm", name="predps")
            nc.tensor.matmul(ps[:1, :w], lhsT=predw[:], rhs=h2[:, s:e],
                             start=True, stop=True)
            nc.vector.tensor_scalar(outsb[:, s:e], ps[:1, :w], predb[:],
                                    None, ALU.add)
        nc.sync.dma_start(out=out_d[:], in_=outsb[:])
        late.release()
        ps_late.release()

    nc.compile()
    return nc


# ---------------------------------------------------------------------------
# entry point
# ---------------------------------------------------------------------------

_prog_cache = {}


def kernel(**inputs) -> np.ndarray:
    cfg, in_maps = host_prep(inputs)
    key = cfg.key()
    if key not in _prog_cache:
        _prog_cache[key] = build_program(cfg)
    nc = _prog_cache[key]
    res = run_bass_kernel_spmd(nc, in_maps, list(range(NCORES)))
    out = np.asarray(res.results[0]['out']).reshape(-1)[:cfg.C]
    return out.astype(np.float32)
